# revision 3
# baseline (speedup 1.0000x reference)
"""GAT (4-layer, 8-head) + GraphNorm kernel for 8 TRN2 NeuronCores.

v2: destination-sharded message passing with a decoupled, unit-granular
gather stream. Each core owns N/8 nodes and all edges pointing at them.
Per layer, a table of per-node rows z = h @ (Wg @ blockdiag(T_h)) (bf16,
256B rows) is replicated via AllGather; T_h's first column equals
att_src[h] so the per-edge source attention term is z[16h] for free.

Key properties vs v1:
- nodes are globally degree-sorted per core (tighter slot grids);
- self-loops never hit DRAM: each core keeps its own z rows in an SBUF
  arena and folds the self term in locally (-12% gather rows);
- gathers are issued per 8-slot unit into a deep SBUF arena pool so the
  Pool engine streams descriptors ~2 blocks ahead without stalling;
- aggregation runs feature-major (out[f, j]) so the Tinv unrotation needs
  no extra transpose; GraphNorm stats reuse one transpose per block.
"""

import sys

import numpy as np

if "/opt/trn_rl_repo" not in sys.path:
    sys.path.insert(0, "/opt/trn_rl_repo")

# ---------------------------------------------------------------- config

N_CORES = 8
H = 8
DH = 16
HID = 128
EPS = 1e-5
NEG_SLOPE = 0.2
PAD_ES = -80.0  # es value stored in the pad table row; kills pad-slot alphas

FULL_CFG = dict(N=50000, E=800000, G=16, L=4)


def _derive(cfg):
    N = cfg["N"]
    npc = N // N_CORES                      # real nodes per core
    nblk = (npc + 127) // 128               # 128-node blocks per core
    npad = nblk * 128                       # padded nodes per core
    trows = N_CORES * npad                  # global table rows
    tbase = max(0, trows - 32767)           # gather base row (neg-idx trick)
    assert npc < npad, "need at least one pad node for the pad table row"
    pad_row = trows - 1                     # last pad node of core 7
    assert pad_row - tbase >= 0
    return dict(NPC=npc, NBLK=nblk, NPAD=npad, TROWS=trows, TBASE=tbase,
                PAD_ROW=pad_row, **cfg)


# ---------------------------------------------------------- host preprocess


def _padrow():
    import ml_dtypes
    r = np.zeros((1, HID), np.float32)
    r[0, 0::DH] = PAD_ES
    return r.astype(ml_dtypes.bfloat16)


def _preprocess(inputs, cfg):
    """All numpy. Returns per-core data + SPMD-uniform shape info."""
    d = _derive(cfg)
    N, G, L = d["N"], d["G"], d["L"]
    NPC, NBLK, NPAD, TBASE, PAD_ROW = (
        d["NPC"], d["NBLK"], d["NPAD"], d["TBASE"], d["PAD_ROW"])

    x = np.asarray(inputs["x"], np.float32)
    ei = np.asarray(inputs["edge_index"], np.int64)
    batch = np.asarray(inputs["batch"], np.int64).astype(np.int32)
    # self-loops are handled locally on-device; grids hold raw edges only
    src = ei[0].astype(np.int64)
    dst = ei[1].astype(np.int64)

    # ---- per-core node permutation: global degree-sort (desc)
    deg_all = np.bincount(dst, minlength=N)          # in-degree w/o self loop
    perms = []          # perms[c][new_pos] = orig local id
    for c in range(N_CORES):
        lo, hi = c * NPC, (c + 1) * NPC
        order = np.argsort(-deg_all[lo:hi], kind="stable")
        perms.append(order.astype(np.int64))
    inv_perms = [np.argsort(p) for p in perms]

    row_of = np.empty(N, np.int64)
    for c in range(N_CORES):
        row_of[c * NPC:(c + 1) * NPC] = c * NPAD + inv_perms[c]

    # ---- per-core edge slot grids
    # block max degrees, SPMD-uniform: max over cores per block index
    deg_perm = [deg_all[c * NPC:(c + 1) * NPC][perms[c]] for c in range(N_CORES)]
    d_i = np.zeros(NBLK, np.int64)
    for c in range(N_CORES):
        dp = np.zeros(NPAD, np.int64)
        dp[:NPC] = deg_perm[c]
        d_i = np.maximum(d_i, dp.reshape(NBLK, 128).max(1))
    d_i = np.maximum(d_i, 1)

    pad_idx = PAD_ROW - TBASE
    core_edges = []     # per core: (dloc, src) arrays
    for c in range(N_CORES):
        m = (dst >= c * NPC) & (dst < (c + 1) * NPC)
        s_c = src[m]
        dloc = inv_perms[c][dst[m] - c * NPC]     # permuted local pos
        core_edges.append((dloc, s_c))

    # uniform gather-unit split per block: units of up to 8 slot-cols
    units = []          # list of (blk, col0, ncols) — identical across cores
    for i in range(NBLK):
        s0 = 0
        while s0 < d_i[i]:
            nc_ = min(8, d_i[i] - s0)
            units.append((i, s0, int(nc_)))
            s0 += nc_
    idx_w = sum(u[2] * 8 for u in units)          # int16 cols in wrapped layout

    idx_arrs = []
    for c in range(N_CORES):
        dloc, s_c = core_edges[c]
        grid = np.full((NBLK, 128, int(d_i.max())), pad_idx, np.int64)
        order = np.argsort(dloc, kind="stable")
        dloc_s, src_s = dloc[order], s_c[order]
        slot = np.zeros(len(dloc_s), np.int64)
        if len(dloc_s):
            new_node = np.r_[True, dloc_s[1:] != dloc_s[:-1]]
            idx0 = np.flatnonzero(new_node)
            counts = np.diff(np.r_[idx0, len(dloc_s)])
            slot = np.arange(len(dloc_s)) - np.repeat(idx0, counts)
        blk = dloc_s // 128
        j = dloc_s % 128
        grid[blk, j, slot] = row_of[src_s] - TBASE
        assert grid.min() >= -32768 and grid.max() <= 32767

        # tail-strip safety: last idx of every gather unit must be >= 0.
        for (i, s0, ncl) in units:
            last_col = s0 + ncl - 1
            if grid[i, 127, last_col] < 0:
                row = grid[i, 127, s0:s0 + ncl]
                cand = np.flatnonzero(row >= 0)
                if len(cand):
                    k = cand[0]
                    row[ncl - 1], row[k] = row[k], row[ncl - 1]
                else:
                    # node 127's slots in this unit are all negative-idx
                    # (valid rows of cores 0-2). Swap the whole unit row of
                    # node 127 with a node whose last element is >= 0; slot
                    # order within a node's list is free, and the swap is
                    # WITHIN the same unit columns so other units unaffected.
                    done = False
                    for j2 in range(127):
                        r2 = grid[i, j2, s0:s0 + ncl]
                        if r2[ncl - 1] >= 0:
                            tmp = r2.copy()
                            grid[i, j2, s0:s0 + ncl] = row
                            grid[i, 127, s0:s0 + ncl] = tmp
                            done = True
                            break
                    if not done:
                        raise RuntimeError("cannot fix tail-strip")
        # wrapped int16 layout per unit: flat i -> [i%16, i//16], 8x replicated
        parts = []
        for (i, s0, ncl) in units:
            flat = grid[i, :, s0:s0 + ncl].T.reshape(-1)   # slot-major: s*128+j
            w = flat.reshape(-1, 16).T                     # [16, n/16]
            parts.append(np.tile(w, (8, 1)))
        idx_arrs.append(np.concatenate(parts, axis=1).astype(np.int16))

    # ---- graph one-hot tiles per core per block
    g1h = []
    g1ht = []
    cnt = np.bincount(batch, minlength=G).astype(np.float64)
    for c in range(N_CORES):
        bperm = batch[c * NPC:(c + 1) * NPC][perms[c]]
        gm = np.zeros((NPAD, G), np.float32)
        gm[np.arange(NPC), bperm] = 1.0
        gmb = gm.reshape(NBLK, 128, G)
        # g1h: [128 j, NBLK*G] — block i's one-hot at cols [i*G, (i+1)*G)
        g1h.append(np.ascontiguousarray(
            gmb.transpose(1, 0, 2).reshape(128, NBLK * G)))
        # g1ht: [G, NBLK*128] — col (i*128 + j) = membership of node (i, j)
        g1ht.append(np.ascontiguousarray(gm.T))
    # x transposed + permuted + padded
    xT = []
    for c in range(N_CORES):
        xp = np.zeros((NPAD, x.shape[1]), np.float32)
        xp[:NPC] = x[c * NPC:(c + 1) * NPC][perms[c]]
        xT.append(np.ascontiguousarray(xp.T))

    # ---- weights
    in_W = np.asarray(inputs["in_W"], np.float32)
    in_b = np.asarray(inputs["in_b"], np.float32)
    Wg = np.asarray(inputs["Wg"], np.float32)
    att_src = np.asarray(inputs["att_src"], np.float32)
    att_dst = np.asarray(inputs["att_dst"], np.float32)
    gat_b = np.asarray(inputs["gat_b"], np.float32)
    gn_w = np.asarray(inputs["gn_w"], np.float32)
    gn_b = np.asarray(inputs["gn_b"], np.float32)
    gn_s = np.asarray(inputs["gn_s"], np.float32)

    W_z = np.zeros((L, HID, HID), np.float32)
    W_ed = np.zeros((L, HID, H), np.float32)
    Tinv_bd = np.zeros((L, HID, HID), np.float32)
    for l in range(L):
        for h in range(H):
            a = att_src[l, h]                          # [16]
            rng = np.random.default_rng(1234 + l * 16 + h)
            M = np.concatenate([a[:, None],
                                rng.standard_normal((DH, DH - 1))], 1)
            q, _ = np.linalg.qr(M)
            T = np.concatenate([a[:, None], q[:, 1:]], 1)  # [16,16]
            Ti = np.linalg.inv(T)
            sl = slice(h * DH, (h + 1) * DH)
            W_z[l][:, sl] = Wg[l][:, sl] @ T
            Tinv_bd[l][sl, sl] = Ti
            W_ed[l][:, h] = Wg[l][:, sl] @ att_dst[l, h]

    cnt_recip = np.zeros(G, np.float32)
    nz = cnt > 0
    cnt_recip[nz] = (1.0 / cnt[nz]).astype(np.float32)

    s = gn_s  # [L, HID]
    s2c = 2.0 * s - s * s                                  # (2s - s^2) per f

    headmap = np.zeros((H, HID), np.float32)               # expand [H]->[HID]
    for h in range(H):
        headmap[h, h * DH:(h + 1) * DH] = 1.0

    consts = dict(
        inw=in_W,                                          # [F_in, 128]
        inb=in_b.reshape(HID, 1),                          # [128,1]
        wz=W_z, wed=W_ed, tinv=Tinv_bd,
        gatb=np.ascontiguousarray(gat_b.T),                # [128, L]
        gnw=np.ascontiguousarray(gn_w.T),                  # [128, L]
        gnb=np.ascontiguousarray(gn_b.T),
        gns=np.ascontiguousarray(s.T),
        gns2c=np.ascontiguousarray(s2c.T),
        cntr=np.tile(cnt_recip[None, :], (HID, 1)),        # [128, G]
        ident=np.eye(HID, dtype=np.float32),
        headmap=headmap,
        padrow=_padrow(),
    )

    return dict(d=d, units=units, d_i=d_i, idx_w=idx_w,
                idx_arrs=idx_arrs, g1h=g1h, g1ht=g1ht, xT=xT,
                perms=perms, inv_perms=inv_perms, consts=consts,
                batch=batch)


# ------------------------------------------------- numpy device emulation
# Mirrors the device program exactly (layouts, pads, bf16 rounding at the
# table) so host logic can be validated without a compile.


def _bf16(a):
    import ml_dtypes
    return a.astype(ml_dtypes.bfloat16).astype(np.float32)


def _numpy_pipeline(prep, dbg=None):
    d = prep["d"]
    L, G = d["L"], d["G"]
    NPC, NBLK, NPAD, TROWS, TBASE = (
        d["NPC"], d["NBLK"], d["NPAD"], d["TROWS"], d["TBASE"])
    C = prep["consts"]
    units, d_i = prep["units"], prep["d_i"]

    # input proj (per core, [128 f, NPAD n])
    hT = [C["inw"].T @ prep["xT"][c] + C["inb"] for c in range(N_CORES)]

    for l in range(L):
        # ---- table build + allgather (+ local z rows in bf16)
        tbl = np.zeros((TROWS, HID), np.float32)
        eds = []
        zrows = []
        for c in range(N_CORES):
            zT = C["wz"][l].T @ hT[c]                     # [128, NPAD]
            edT = C["wed"][l].T @ hT[c]                   # [H, NPAD]
            rows = _bf16(zT.T)                            # [NPAD, 128] bf16
            tbl[c * NPAD:(c + 1) * NPAD] = rows
            zrows.append(rows)                            # SBUF z arena
            eds.append(edT)
        for c in range(N_CORES):
            tbl[(c + 1) * NPAD - 1] = 0.0
            tbl[(c + 1) * NPAD - 1, 0::DH] = PAD_ES
        tblv = tbl  # already bf16-rounded

        # ---- edge phase per core
        new_hT = []
        stats = np.zeros((N_CORES, HID, G, 2), np.float32)
        for c in range(N_CORES):
            idx = prep["idx_arrs"][c]
            xt_new = np.zeros((HID, NPAD), np.float32)
            col = 0
            for i in range(NBLK):
                di = int(d_i[i])
                msg = np.zeros((128, di, HID), np.float32)
                s0 = 0
                while s0 < di:
                    ncl = min(8, di - s0)
                    w = idx[:16, col:col + ncl * 8]        # [16, n/16]
                    flat = w.T.reshape(-1)                 # i -> idx
                    col += ncl * 8
                    rows = tblv[flat.astype(np.int64) + TBASE]
                    msg[:, s0:s0 + ncl, :] = (
                        rows.reshape(ncl, 128, HID).transpose(1, 0, 2))
                    s0 += ncl
                es = msg[:, :, 0::DH]                      # [128, di, H]
                ed = eds[c][:, i * 128:(i + 1) * 128].T    # [128, H]
                e = es + ed[:, None, :]
                e = np.maximum(e, NEG_SLOPE * e)
                ex = np.exp(e)                             # [128, di, H]
                exb = _bf16(ex)
                # self-loop term, computed from the local z arena
                zsb = zrows[c][i * 128:(i + 1) * 128]      # [128, HID] bf16
                es_s = zsb[:, 0::DH]                       # [128, H]
                e_s = es_s + ed
                e_s = np.maximum(e_s, NEG_SLOPE * e_s)
                ex_s = np.exp(e_s)                         # [128, H] f32
                exb_s = _bf16(ex_s)
                denom = ex.sum(1) + ex_s                   # [128, H]
                mp = _bf16(msg * exb.repeat(DH, axis=2))
                num = mp.sum(1)                            # [128 j, 128 f]
                num = num + _bf16(zsb * exb_s.repeat(DH, axis=1))
                gat = num * (1.0 / denom).repeat(DH, axis=1)
                attnT = C["tinv"][l].T @ gat.T             # [f', j]
                xt = attnT + hT[c][:, i * 128:(i + 1) * 128] + C["gatb"][:, l:l + 1]
                xt_new[:, i * 128:(i + 1) * 128] = xt
                g1hb = prep["g1h"][c][:, i * G:(i + 1) * G]  # [128 j, G]
                stats[c, :, :, 0] += xt @ g1hb               # sum x: [f, G]
                stats[c, :, :, 1] += (xt * xt) @ g1hb
            new_hT.append(xt_new)

        # ---- allreduce stats + norm
        tot = stats.sum(0)                                 # [f, G, 2]
        mean = tot[:, :, 0] * C["cntr"]
        ex2 = tot[:, :, 1] * C["cntr"]
        var = ex2 - C["gns2c"][:, l:l + 1] * mean * mean
        rstd = 1.0 / np.sqrt(var + EPS)
        c1 = C["gnw"][:, l:l + 1] * rstd                   # [f, G]
        c0 = C["gnb"][:, l:l + 1] - C["gns"][:, l:l + 1] * mean * c1
        for c in range(N_CORES):
            g1ht = prep["g1ht"][c]                         # [G, NBLK*128]
            p1 = c1 @ g1ht                                 # [f, NPAD]
            p0 = c0 @ g1ht
            hT[c] = new_hT[c] * p1 + p0

    # ---- output assembly: hT[c][:, p] holds node perms[c][p]
    N = d["N"]
    out = np.zeros((N, HID), np.float32)
    for c in range(N_CORES):
        out[c * NPC + prep["perms"][c]] = hT[c][:, :NPC].T
    return out


# ---------------------------------------------------------------- device


def _build_program(prep, timing_reps=None, dbg=False, ablate=()):
    import contextlib

    import concourse.tile as tile
    from concourse import bacc, mybir, library_config

    d = prep["d"]
    L, G = d["L"], d["G"]
    NPC, NBLK, NPAD, TROWS, TBASE = (
        d["NPC"], d["NBLK"], d["NPAD"], d["TROWS"], d["TBASE"])
    units, d_i, idx_w = prep["units"], prep["d_i"], prep["idx_w"]
    F_IN = prep["xT"][0].shape[0]
    NCHUNK = (NPAD + 511) // 512
    AF = mybir.ActivationFunctionType
    ALU = mybir.AluOpType

    f32, bf16, i16 = mybir.dt.float32, mybir.dt.bfloat16, mybir.dt.int16

    nc = bacc.Bacc(None, target_bir_lowering=False, num_swdge_queues=4)

    def param(name, shape, dtype=f32, out=False):
        return nc.declare_dram_parameter(name, list(shape), dtype, isOutput=out)

    P = dict(
        xT=param("xT", [F_IN, NPAD]),
        idx=param("idx", [128, idx_w], i16),
        g1h=param("g1h", [128, NBLK * G]),
        g1ht=param("g1ht", [G, NBLK * 128]),
        inw=param("inw", [F_IN, HID]),
        inb=param("inb", [HID, 1]),
        wz=param("wz", [L, HID, HID]),
        wed=param("wed", [L, HID, H]),
        tinv=param("tinv", [L, HID, HID]),
        gatb=param("gatb", [HID, L]),
        gnw=param("gnw", [HID, L]),
        gnb=param("gnb", [HID, L]),
        gns=param("gns", [HID, L]),
        gns2c=param("gns2c", [HID, L]),
        cntr=param("cntr", [HID, G]),
        ident=param("ident", [HID, HID]),
        headmap=param("headmap", [H, HID]),
        padrow=param("padrow", [1, HID], bf16),
        out=param("out", [NPC, HID], out=True),
    )

    qctr = [0]

    def next_q():
        q = qctr[0] % 4
        qctr[0] += 1
        return q

    with tile.TileContext(nc) as tc:
        est = contextlib.ExitStack()
        singles = est.enter_context(tc.tile_pool(name="singles", bufs=1))
        msgp = est.enter_context(tc.tile_pool(name="msg", bufs=20))
        etp = est.enter_context(tc.tile_pool(name="etile", bufs=8))
        exp_ = est.enter_context(tc.tile_pool(name="exb", bufs=20))
        denp = est.enter_context(tc.tile_pool(name="den", bufs=12))
        blkp = est.enter_context(tc.tile_pool(name="blk", bufs=8))
        stag = est.enter_context(tc.tile_pool(name="stag", bufs=3))
        psA = est.enter_context(tc.tile_pool(name="psA", bufs=1, space="PSUM"))
        psnp = est.enter_context(tc.tile_pool(name="psnp", bufs=2, space="PSUM"))
        pssm = est.enter_context(tc.tile_pool(name="pssm", bufs=3, space="PSUM"))
        psacc = est.enter_context(tc.tile_pool(name="psacc", bufs=1, space="PSUM"))
        dram = est.enter_context(tc.tile_pool(name="dram", bufs=1, space="DRAM"))

        nc.gpsimd.load_library(library_config.mlp)

        # ---------------- constants
        def load(t, src):
            nc.sync.dma_start(out=t, in_=src)
            return t

        ident_f = load(singles.tile([HID, HID], f32, name="idf"), P["ident"][:])
        ident_b = singles.tile([HID, HID], bf16, name="idb")
        nc.vector.tensor_copy(out=ident_b, in_=ident_f)
        headmap_sb = load(singles.tile([H, HID], f32, name="hm"), P["headmap"][:])
        idx_sb = load(singles.tile([128, idx_w], i16, name="idxs"), P["idx"][:])
        g1h_sb = load(singles.tile([128, NBLK * G], f32, name="g1h"), P["g1h"][:])
        inw_sb = load(singles.tile([F_IN, HID], f32, name="inw"), P["inw"][:])
        inb_sb = load(singles.tile([HID, 1], f32, name="inb"), P["inb"][:])
        wz_sb = [load(singles.tile([HID, HID], f32, name=f"wz{l}"), P["wz"][l])
                 for l in range(L)]
        wed_sb = [load(singles.tile([HID, H], f32, name=f"wed{l}"), P["wed"][l])
                  for l in range(L)]
        tinv_sb = [load(singles.tile([HID, HID], f32, name=f"ti{l}"), P["tinv"][l])
                   for l in range(L)]
        gatb_sb = load(singles.tile([HID, L], f32, name="gatb"), P["gatb"][:])
        gnw_sb = load(singles.tile([HID, L], f32, name="gnw"), P["gnw"][:])
        gnb_sb = load(singles.tile([HID, L], f32, name="gnb"), P["gnb"][:])
        gns_sb = load(singles.tile([HID, L], f32, name="gns"), P["gns"][:])
        gns2c_sb = load(singles.tile([HID, L], f32, name="gns2c"), P["gns2c"][:])
        cntr_sb = load(singles.tile([HID, G], f32, name="cntr"), P["cntr"][:])
        eps_sb = singles.tile([HID, 1], f32, name="eps")
        nc.vector.memset(eps_sb, EPS)

        h_a = singles.tile([HID, NPAD], f32, name="h_a")
        ed_all = singles.tile([128, NBLK * H], f32, name="ed_all")
        z_arena = singles.tile([128, NBLK * HID], bf16, name="z_arena")

        tbl_in = [dram.tile([NPAD, HID], bf16, name=f"tin{l}") for l in range(L)]
        tbl = [dram.tile([TROWS, HID], bf16, addr_space="Shared", name=f"tbl{l}")
               for l in range(L)]
        st_in = [dram.tile([HID, 2 * G], f32, name=f"st_in{l}")
                 for l in range(L)]
        st_out = [dram.tile([HID, 2 * G], f32, addr_space="Shared",
                            name=f"st_out{l}") for l in range(L)]

        # ---------------- input projection: h0^T = inw^T @ x^T + b
        for k in range(NCHUNK):
            c0, c1_ = k * 512, min((k + 1) * 512, NPAD)
            w = c1_ - c0
            xt = stag.tile([F_IN, 512], f32, name="xchunk")
            nc.sync.dma_start(out=xt[:, :w], in_=P["xT"][:, c0:c1_])
            ps = psA.tile([HID, 512], f32, name="psbig")
            nc.tensor.matmul(out=ps[:, :w], lhsT=inw_sb, rhs=xt[:, :w],
                             start=True, stop=True)
            nc.scalar.activation(out=h_a[:, c0:c1_], in_=ps[:, :w],
                                 func=AF.Identity, bias=inb_sb[:, 0:1])

        # ---------------- gather table build for layer l from h_src
        def build_table(l, h_src):
            for k in range(NCHUNK):
                c0, c1_ = k * 512, min((k + 1) * 512, NPAD)
                w = c1_ - c0
                psz = psA.tile([HID, 512], f32, name="psbig")
                nc.tensor.matmul(out=psz[:, :w], lhsT=wz_sb[l],
                                 rhs=h_src[:, c0:c1_], start=True, stop=True)
                zt = stag.tile([HID, 512], f32, name="zt")
                nc.scalar.activation(out=zt[:, :w], in_=psz[:, :w], func=AF.Copy)
                pse = psA.tile([HID, 512], f32, name="psbig")
                nc.tensor.matmul(out=pse[:H, :w], lhsT=wed_sb[l],
                                 rhs=h_src[:, c0:c1_], start=True, stop=True)
                edt = stag.tile([H, 512], f32, name="edt")
                nc.vector.tensor_copy(out=edt[:, :w], in_=pse[:H, :w])
                for bb in range(w // 128):
                    i = (c0 + bb * 128) // 128
                    pst = pssm.tile([128, HID], f32, name="pssm")
                    nc.tensor.matmul(
                        out=pst, lhsT=zt[:, bb * 128:(bb + 1) * 128],
                        rhs=ident_f, start=True, stop=True)
                    zsl = z_arena[:, i * HID:(i + 1) * HID]
                    nc.scalar.activation(out=zsl, in_=pst, func=AF.Copy)
                    nc.sync.dma_start(out=tbl_in[l][i * 128:(i + 1) * 128, :],
                                      in_=zsl)
                    if i == NBLK - 1:
                        nc.sync.dma_start(
                            out=tbl_in[l][NPAD - 1:NPAD, :],
                            in_=P["padrow"][:])
                    psd = pssm.tile([128, HID], f32, name="pssm")
                    nc.tensor.matmul(
                        out=psd[:, :H], lhsT=edt[:, bb * 128:(bb + 1) * 128],
                        rhs=ident_f[:H, :H], start=True, stop=True)
                    nc.vector.tensor_copy(out=ed_all[:, i * H:(i + 1) * H],
                                          in_=psd[:, :H])
            if timing_reps is None:
                nc.gpsimd.collective_compute(
                    "AllGather", mybir.AluOpType.bypass,
                    replica_groups=[list(range(N_CORES))],
                    ins=[tbl_in[l].opt()], outs=[tbl[l].opt()])
            else:
                nc.sync.dma_start(out=tbl[l][:NPAD, :], in_=tbl_in[l][:])

        # ---------------- edge phase: h_src -> h_dst (pre-norm x^T)
        # Unit-granular software pipeline: gathers stream LA units ahead of
        # the attention/aggregation consumers so the Pool engine never waits
        # on compute. Block finalization (self term, normalize, unrotate,
        # residual, stats) fires after a block's last unit is consumed.
        stats1 = psacc.tile([HID, G], f32, name="sa")
        stats2 = psacc.tile([HID, G], f32, name="sb")

        def edge_phase(l, h_src, h_dst):
            ucol = {}
            col = 0
            for (i, s0, ncl) in units:
                ucol[(i, s0)] = col
                col += ncl * 8
            NU = len(units)
            LA = 14                     # gather lookahead (units)
            state = {}                  # u -> (msg, exb) after stageB issue
            bden = {}                   # block -> list of den tiles
            bnps = {}                   # block -> PSUM accumulator

            def stageA(u):              # issue gathers for unit u
                (i, s0, ncl) = units[u]
                msg = msgp.tile([128, 8, HID], bf16, name="msg")
                c0 = ucol[(i, s0)]
                nidx = ncl * 128
                if "gather" not in ablate:
                    nc.gpsimd.dma_gather(
                        out_ap=msg[:, :ncl, :],
                        in_ap=tbl[l][TBASE:, :],
                        idxs_ap=idx_sb[:, c0:c0 + ncl * 8],
                        num_idxs=nidx, num_idxs_reg=nidx, elem_size=HID,
                        queue_num=next_q())
                else:
                    nc.vector.memset(msg[:, :ncl, :], 0.25)
                state[u] = msg

            def stageB(u):              # attention + weight + aggregate
                (i, s0, ncl) = units[u]
                msg = state.pop(u)
                e_t = etp.tile([128, 8, H], f32, name="e_t")
                nc.vector.tensor_tensor(
                    out=e_t[:, :ncl, :],
                    in0=msg[:, :ncl, 0:HID:DH],
                    in1=ed_all[:, i * H:(i + 1) * H].unsqueeze(1)
                        .to_broadcast([128, ncl, H]),
                    op=ALU.add)
                e_s = etp.tile([128, 8, H], f32, name="e_s")
                nc.vector.tensor_scalar_mul(
                    e_s[:, :ncl, :], e_t[:, :ncl, :], NEG_SLOPE)
                nc.vector.tensor_tensor(
                    out=e_t[:, :ncl, :], in0=e_t[:, :ncl, :],
                    in1=e_s[:, :ncl, :], op=ALU.max)
                exb = exp_.tile([128, 8, H], bf16, name="exb")
                nc.scalar.activation(out=exb[:, :ncl, :], in_=e_t[:, :ncl, :],
                                     func=AF.Exp)
                den = denp.tile([128, H], f32, name="den")
                nc.vector.tensor_reduce(
                    out=den, in_=exb[:, :ncl, :].rearrange("p a b -> p b a"),
                    axis=mybir.AxisListType.X, op=ALU.add)
                bden.setdefault(i, []).append(den)
                if "post" in ablate:
                    return
                nc.vector.tensor_tensor(
                    out=msg[:, :ncl, :].rearrange("p a (b c) -> p a b c", b=H),
                    in0=msg[:, :ncl, :].rearrange("p a (b c) -> p a b c", b=H),
                    in1=exb[:, :ncl, :].unsqueeze(3)
                        .to_broadcast([128, ncl, H, DH]),
                    op=ALU.mult)
                if i not in bnps:
                    bnps[i] = psnp.tile([128, HID], f32, name="npsT")
                first = (s0 == 0)
                for sj in range(ncl):
                    # feature-major accumulation: out[f, j] += msg[j, sj, f]^T
                    nc.tensor.matmul(out=bnps[i], lhsT=msg[:, sj, :],
                                     rhs=ident_b,
                                     start=(first and sj == 0), stop=False)

            def stageC(i):              # block finalize
                sl = slice(i * 128, (i + 1) * 128)
                zsl = z_arena[:, i * HID:(i + 1) * HID]
                # self-loop attention from the local z arena
                es_t = etp.tile([128, H], f32, name="es_t")
                nc.vector.tensor_tensor(
                    out=es_t, in0=zsl[:, 0:HID:DH],
                    in1=ed_all[:, i * H:(i + 1) * H], op=ALU.add)
                es_m = etp.tile([128, H], f32, name="es_m")
                nc.vector.tensor_scalar_mul(es_m, es_t, NEG_SLOPE)
                nc.vector.tensor_tensor(out=es_t, in0=es_t, in1=es_m,
                                        op=ALU.max)
                ex_f = denp.tile([128, H], f32, name="ex_f")
                nc.scalar.activation(out=ex_f, in_=es_t, func=AF.Exp)
                ex_b = exp_.tile([128, H], bf16, name="ex_b")
                nc.vector.tensor_copy(out=ex_b, in_=ex_f)
                if "post" in ablate:
                    nc.vector.tensor_copy(
                        out=h_dst[:, i * 128:i * 128 + 8], in_=ex_f)
                    return
                tmps = blkp.tile([128, HID], bf16, name="tmps")
                nc.vector.tensor_tensor(
                    out=tmps.rearrange("p (a b) -> p a b", a=H),
                    in0=zsl.rearrange("p (a b) -> p a b", a=H),
                    in1=ex_b.unsqueeze(2).to_broadcast([128, H, DH]),
                    op=ALU.mult)
                nps = bnps.pop(i)
                nc.tensor.matmul(out=nps, lhsT=tmps, rhs=ident_b,
                                 start=False, stop=True)
                # denominator: sum unit partials + self
                dens = bden.pop(i)
                dacc = denp.tile([128, H], f32, name="dacc")
                nc.vector.tensor_tensor(out=dacc, in0=dens[0], in1=ex_f,
                                        op=ALU.add)
                for dn in dens[1:]:
                    nc.vector.tensor_tensor(out=dacc, in0=dacc, in1=dn,
                                            op=ALU.add)
                rec = denp.tile([128, H], f32, name="rec")
                nc.vector.reciprocal(out=rec, in_=dacc)
                # recT expanded to [f, j]
                prt = pssm.tile([128, HID], f32, name="pssm")
                nc.tensor.matmul(out=prt[:H, :], lhsT=rec, rhs=ident_f,
                                 start=True, stop=True)
                rt = blkp.tile([H, HID], f32, name="rt")
                nc.vector.tensor_copy(out=rt, in_=prt[:H, :])
                pre = pssm.tile([128, HID], f32, name="pssm")
                nc.tensor.matmul(out=pre, lhsT=headmap_sb, rhs=rt,
                                 start=True, stop=True)
                rex = blkp.tile([128, HID], f32, name="rex")
                nc.vector.tensor_copy(out=rex, in_=pre)
                # normalize numerator (feature-major), then unrotate
                numT = blkp.tile([128, HID], f32, name="numT")
                nc.vector.tensor_tensor(out=numT, in0=nps, in1=rex,
                                        op=ALU.mult)
                pat = pssm.tile([128, HID], f32, name="pssm")
                nc.tensor.matmul(out=pat, lhsT=tinv_sb[l], rhs=numT,
                                 start=True, stop=True)
                xt = blkp.tile([128, HID], f32, name="xt")
                nc.scalar.activation(out=xt, in_=pat, func=AF.Identity,
                                     bias=gatb_sb[:, l:l + 1])
                nc.vector.tensor_tensor(out=h_dst[:, sl], in0=xt,
                                        in1=h_src[:, sl], op=ALU.add)
                if "stats" in ablate:
                    return
                pxb = pssm.tile([128, HID], f32, name="pssm")
                nc.tensor.matmul(out=pxb, lhsT=h_dst[:, sl], rhs=ident_f,
                                 start=True, stop=True)
                xb = blkp.tile([128, HID], f32, name="xb")
                nc.vector.tensor_copy(out=xb, in_=pxb)
                sq = blkp.tile([128, HID], f32, name="sq")
                nc.scalar.activation(out=sq, in_=xb, func=AF.Square)
                nc.tensor.matmul(out=stats1, lhsT=xb,
                                 rhs=g1h_sb[:, i * G:(i + 1) * G],
                                 start=(i == 0), stop=(i == NBLK - 1),
                                 skip_group_check=True)
                nc.tensor.matmul(out=stats2, lhsT=sq,
                                 rhs=g1h_sb[:, i * G:(i + 1) * G],
                                 start=(i == 0), stop=(i == NBLK - 1),
                                 skip_group_check=True)

            NU_list = units
            for k in range(NU + LA):
                if k < NU:
                    stageA(k)
                kb = k - LA
                if 0 <= kb < NU:
                    stageB(kb)
                    (ib, s0b, nclb) = NU_list[kb]
                    if kb == NU - 1 or NU_list[kb + 1][0] != ib:
                        stageC(ib)

            stl = stag.tile([HID, 2 * G], f32, name="stl")
            if "post" in ablate or "stats" in ablate:
                nc.vector.memset(stl, 1.0)
            else:
                nc.vector.tensor_copy(out=stl[:, :G], in_=stats1)
                nc.vector.tensor_copy(out=stl[:, G:], in_=stats2)
            nc.sync.dma_start(out=st_in[l], in_=stl)
            if timing_reps is None:
                nc.gpsimd.collective_compute(
                    "AllReduce", mybir.AluOpType.add,
                    replica_groups=[list(range(N_CORES))],
                    ins=[st_in[l].opt()], outs=[st_out[l].opt()])
            else:
                nc.sync.dma_start(out=st_out[l][:], in_=st_in[l][:])
            stg = stag.tile([HID, 2 * G], f32, name="stg")
            nc.sync.dma_start(out=stg, in_=st_out[l])
            return stg

        # ---------------- graph norm applied to h (in place)
        def norm_apply(l, stg, h):
            mean = blkp.tile([128, G], f32, name="mean")
            nc.vector.tensor_tensor(out=mean, in0=stg[:, :G], in1=cntr_sb,
                                    op=ALU.mult)
            ex2 = blkp.tile([128, G], f32, name="ex2")
            nc.vector.tensor_tensor(out=ex2, in0=stg[:, G:], in1=cntr_sb,
                                    op=ALU.mult)
            m2 = blkp.tile([128, G], f32, name="m2")
            nc.vector.tensor_tensor(out=m2, in0=mean, in1=mean, op=ALU.mult)
            nc.vector.tensor_tensor(
                out=m2, in0=m2,
                in1=gns2c_sb[:, l:l + 1].to_broadcast([HID, G]),
                op=ALU.mult)
            var = blkp.tile([128, G], f32, name="var")
            nc.vector.tensor_tensor(out=var, in0=ex2, in1=m2, op=ALU.subtract)
            nc.scalar.activation(out=var, in_=var, func=AF.Sqrt, bias=eps_sb[:, 0:1])
            rstd = blkp.tile([128, G], f32, name="rstd")
            nc.vector.reciprocal(out=rstd, in_=var)
            c1 = blkp.tile([128, G], f32, name="c1")
            nc.vector.tensor_tensor(
                out=c1, in0=rstd,
                in1=gnw_sb[:, l:l + 1].to_broadcast([HID, G]), op=ALU.mult)
            c0t = blkp.tile([128, G], f32, name="c0t")
            nc.vector.tensor_tensor(out=c0t, in0=mean, in1=c1, op=ALU.mult)
            nc.vector.tensor_tensor(
                out=c0t, in0=c0t,
                in1=gns_sb[:, l:l + 1].to_broadcast([HID, G]), op=ALU.mult)
            c0 = blkp.tile([128, G], f32, name="c0")
            nc.vector.tensor_tensor(
                out=c0, in0=gnb_sb[:, l:l + 1].to_broadcast([HID, G]),
                in1=c0t, op=ALU.subtract)
            pc = pssm.tile([128, HID], f32, name="pssm")
            nc.tensor.matmul(out=pc[:G, :], lhsT=c1, rhs=ident_f,
                             start=True, stop=True)
            c1T = blkp.tile([G, HID], f32, name="c1T")
            nc.vector.tensor_copy(out=c1T, in_=pc[:G, :])
            pc2 = pssm.tile([128, HID], f32, name="pssm")
            nc.tensor.matmul(out=pc2[:G, :], lhsT=c0, rhs=ident_f,
                             start=True, stop=True)
            c0T = blkp.tile([G, HID], f32, name="c0T")
            nc.vector.tensor_copy(out=c0T, in_=pc2[:G, :])
            for i in range(NBLK):
                g1htb = stag.tile([G, 128], f32, name="g1htb")
                nc.sync.dma_start(out=g1htb,
                                  in_=P["g1ht"][:, i * 128:(i + 1) * 128])
                p1 = pssm.tile([128, HID], f32, name="pssm")
                nc.tensor.matmul(out=p1, lhsT=c1T, rhs=g1htb,
                                 start=True, stop=True)
                p0 = pssm.tile([128, HID], f32, name="pssm")
                nc.tensor.matmul(out=p0, lhsT=c0T, rhs=g1htb,
                                 start=True, stop=True)
                sl = slice(i * 128, (i + 1) * 128)
                nc.vector.tensor_tensor(out=h[:, sl], in0=h[:, sl], in1=p1,
                                        op=ALU.mult)
                nc.vector.tensor_tensor(out=h[:, sl], in0=h[:, sl], in1=p0,
                                        op=ALU.add)

        # ---------------- layers
        h_src = h_dst = h_a
        loop_ctx = tc.For_i(0, timing_reps) if timing_reps else None
        if loop_ctx:
            loop_ctx.__enter__()
        for l in range(L):
            build_table(l, h_src)
            stg = edge_phase(l, h_src, h_dst)
            norm_apply(l, stg, h_dst)
        if loop_ctx:
            loop_ctx.__exit__(None, None, None)

        # ---------------- output rows
        for i in range(NBLK):
            r0 = i * 128
            r1 = min(r0 + 128, NPC)
            if r1 <= r0:
                break
            po = pssm.tile([128, HID], f32, name="pssm")
            nc.tensor.matmul(out=po, lhsT=h_src[:, r0:r0 + 128], rhs=ident_f,
                             start=True, stop=True)
            rows = stag.tile([128, HID], f32, name="orow")
            nc.vector.tensor_copy(out=rows, in_=po)
            nc.sync.dma_start(out=P["out"][r0:r1, :], in_=rows[:r1 - r0, :])

        est.close()

    nc.compile()
    return nc


def _make_inmaps(prep):
    C = prep["consts"]
    maps = []
    for c in range(N_CORES):
        m = dict(
            xT=prep["xT"][c],
            idx=prep["idx_arrs"][c],
            g1h=prep["g1h"][c],
            g1ht=prep["g1ht"][c],
            inw=C["inw"], inb=C["inb"], wz=C["wz"], wed=C["wed"],
            tinv=C["tinv"], gatb=C["gatb"], gnw=C["gnw"], gnb=C["gnb"],
            gns=C["gns"], gns2c=C["gns2c"], cntr=C["cntr"], ident=C["ident"],
            headmap=C["headmap"], padrow=C["padrow"],
        )
        maps.append(m)
    return maps


def _assemble(prep, results):
    d = prep["d"]
    NPC = d["NPC"]
    out = np.zeros((d["N"], HID), np.float32)
    for c in range(N_CORES):
        out[c * NPC + prep["perms"][c]] = results[c]["out"]
    return out


def _run(inputs, cfg):
    from concourse.bass_utils import run_bass_kernel_spmd
    prep = _preprocess(inputs, cfg)
    nc = _build_program(prep)
    res = run_bass_kernel_spmd(nc, _make_inmaps(prep),
                               core_ids=list(range(N_CORES)))
    return _assemble(prep, res.results)


def kernel(**inputs):
    return _run(inputs, FULL_CFG)
